# revision 1
# baseline (speedup 1.0000x reference)
"""MPNN-GGNN forward on 8 Trainium2 NeuronCores.

Data-parallel over the batch: 8 graphs per core. All weights replicated.
Per-core Bass/Tile kernel computes 4 message-passing + GRU steps and the
gated readout entirely on-chip; f32r (full fp32 bits, reduced-precision
multiplier) matmuls at full PE rate.

Layout conventions per core (G = 8 graphs, N = 128 nodes, H = MSG = 512):
  h_all  [128(node), G, 512]  fp32   node-major hidden state
  hT_all [128(feat), G, 512?] -- actually [128, G, 512] where the free
         slice [g, hc*128:(hc+1)*128] holds chunk hc of h^T (feat rows)
  mask_sb[128(w),   G, L, 128(v)]    (e^T == l+1) one-hot adjacency
  matmul convention: out[i,j] = sum_k lhsT[k,i] * rhs[k,j]
"""

import numpy as np

import concourse.mybir as mybir
import concourse.tile as tile
from concourse import bacc
from concourse.bass_utils import run_bass_kernel_spmd

# problem constants (hardcoded per contract)
B, N, F_IN = 64, 128, 128
H, MSG, L = 512, 512, 4
NSTEP = 4
TARGET = 12
NCORES = 8
G = B // NCORES          # graphs per core
HC = H // 128            # h chunks
MC = MSG // 128          # msg chunks
FB = 2                   # readout free blocks (4 graphs x 128 nodes each)
GPB = G // FB

f32 = mybir.dt.float32
f32r = mybir.dt.float32r
AF = mybir.ActivationFunctionType
ALU = mybir.AluOpType
AX = mybir.AxisListType

_CACHE = {}


def _build():
    nc = bacc.Bacc("TRN2", target_bir_lowering=False)

    # ---- DRAM I/O ----
    d_h0 = nc.dram_tensor("h0", [N, G, H], f32, kind="ExternalInput")
    d_hT0 = nc.dram_tensor("hT0", [F_IN, G, N], f32r, kind="ExternalInput")
    d_mask = nc.dram_tensor("mask", [N, G, L, N], f32r, kind="ExternalInput")
    d_A = nc.dram_tensor("A", [128, L, HC, MSG], f32r, kind="ExternalInput")
    d_wih = nc.dram_tensor("wih", [128, MC, 3 * H], f32r, kind="ExternalInput")
    d_whh = nc.dram_tensor("whh", [128, HC, 3 * H], f32r, kind="ExternalInput")
    d_brz = nc.dram_tensor("brz", [1, 2 * H], f32r, kind="ExternalInput")
    d_bin = nc.dram_tensor("bin", [1, H], f32r, kind="ExternalInput")
    d_bhn = nc.dram_tensor("bhn", [1, H], f32r, kind="ExternalInput")
    d_ones = nc.dram_tensor("ones", [1, N], f32r, kind="ExternalInput")
    d_ones12 = nc.dram_tensor("ones12", [1, TARGET], f32r, kind="ExternalInput")
    d_onescol = nc.dram_tensor("onescol", [128, 1], f32r, kind="ExternalInput")
    d_ident = nc.dram_tensor("ident", [128, 128], f32, kind="ExternalInput")
    d_r1w0 = nc.dram_tensor("r1w0", [128, 5, 128], f32r, kind="ExternalInput")
    d_r1w1 = nc.dram_tensor("r1w1", [128, 2, 128], f32r, kind="ExternalInput")
    d_r1w2 = nc.dram_tensor("r1w2", [128, 2, 128], f32r, kind="ExternalInput")
    d_r1w3 = nc.dram_tensor("r1w3", [128, TARGET], f32r, kind="ExternalInput")
    d_r2w0 = nc.dram_tensor("r2w0", [128, 4, 128], f32r, kind="ExternalInput")
    d_r2w1 = nc.dram_tensor("r2w1", [128, 2, 128], f32r, kind="ExternalInput")
    d_r2w2 = nc.dram_tensor("r2w2", [128, 2, 128], f32r, kind="ExternalInput")
    d_r2w3 = nc.dram_tensor("r2w3", [128, TARGET], f32r, kind="ExternalInput")
    d_r1b0 = nc.dram_tensor("r1b0", [128, 1], f32, kind="ExternalInput")
    d_r1b1 = nc.dram_tensor("r1b1", [128, 2], f32, kind="ExternalInput")
    d_r1b2 = nc.dram_tensor("r1b2", [128, 1], f32, kind="ExternalInput")
    d_r1b3 = nc.dram_tensor("r1b3", [TARGET, 1], f32, kind="ExternalInput")
    d_r2b0 = nc.dram_tensor("r2b0", [128, 1], f32, kind="ExternalInput")
    d_r2b1 = nc.dram_tensor("r2b1", [128, 2], f32, kind="ExternalInput")
    d_r2b2 = nc.dram_tensor("r2b2", [128, 1], f32, kind="ExternalInput")
    d_r2b3 = nc.dram_tensor("r2b3", [TARGET, 1], f32, kind="ExternalInput")
    d_out = nc.dram_tensor("out", [TARGET, G], f32, kind="ExternalOutput")

    with tile.TileContext(nc) as tc:
        with tc.tile_pool(name="st", bufs=1) as st, \
             tc.tile_pool(name="state", bufs=1) as stt, \
             tc.tile_pool(name="wk", bufs=2) as wk, \
             tc.tile_pool(name="ps", bufs=1, space="PSUM") as ps:

            # ---- static loads ----
            A_sb = st.tile([128, L, HC, MSG], f32r, tag="A")
            nc.sync.dma_start(A_sb[:], d_A[:])
            wih_sb = st.tile([128, MC, 3 * H], f32r, tag="wih")
            nc.sync.dma_start(wih_sb[:], d_wih[:])
            whh_sb = st.tile([128, HC, 3 * H], f32r, tag="whh")
            nc.sync.dma_start(whh_sb[:], d_whh[:])
            mask_sb = st.tile([N, G, L, N], f32r, tag="mask")
            nc.sync.dma_start(mask_sb[:], d_mask[:])
            hT0_sb = st.tile([F_IN, G, N], f32r, tag="hT0")
            nc.sync.dma_start(hT0_sb[:], d_hT0[:])
            brz_sb = st.tile([1, 2 * H], f32r, tag="brz")
            nc.sync.dma_start(brz_sb[:], d_brz[:])
            bin_sb = st.tile([1, H], f32r, tag="bin")
            nc.sync.dma_start(bin_sb[:], d_bin[:])
            bhn_sb = st.tile([1, H], f32r, tag="bhn")
            nc.sync.dma_start(bhn_sb[:], d_bhn[:])
            ones_sb = st.tile([1, N], f32r, tag="ones")
            nc.sync.dma_start(ones_sb[:], d_ones[:])
            ones12_sb = st.tile([1, TARGET], f32r, tag="ones12")
            nc.sync.dma_start(ones12_sb[:], d_ones12[:])
            onescol_sb = st.tile([128, 1], f32r, tag="onescol")
            nc.sync.dma_start(onescol_sb[:], d_onescol[:])
            ident_sb = st.tile([128, 128], f32, tag="ident")
            nc.sync.dma_start(ident_sb[:], d_ident[:])

            r1w0_sb = st.tile([128, 5, 128], f32r, tag="r1w0")
            nc.sync.dma_start(r1w0_sb[:], d_r1w0[:])
            r1w1_sb = st.tile([128, 2, 128], f32r, tag="r1w1")
            nc.sync.dma_start(r1w1_sb[:], d_r1w1[:])
            r1w2_sb = st.tile([128, 2, 128], f32r, tag="r1w2")
            nc.sync.dma_start(r1w2_sb[:], d_r1w2[:])
            r1w3_sb = st.tile([128, TARGET], f32r, tag="r1w3")
            nc.sync.dma_start(r1w3_sb[:], d_r1w3[:])
            r2w0_sb = st.tile([128, 4, 128], f32r, tag="r2w0")
            nc.sync.dma_start(r2w0_sb[:], d_r2w0[:])
            r2w1_sb = st.tile([128, 2, 128], f32r, tag="r2w1")
            nc.sync.dma_start(r2w1_sb[:], d_r2w1[:])
            r2w2_sb = st.tile([128, 2, 128], f32r, tag="r2w2")
            nc.sync.dma_start(r2w2_sb[:], d_r2w2[:])
            r2w3_sb = st.tile([128, TARGET], f32r, tag="r2w3")
            nc.sync.dma_start(r2w3_sb[:], d_r2w3[:])
            r1b0_sb = st.tile([128, 1], f32, tag="r1b0")
            nc.sync.dma_start(r1b0_sb[:], d_r1b0[:])
            r1b1_sb = st.tile([128, 2], f32, tag="r1b1")
            nc.sync.dma_start(r1b1_sb[:], d_r1b1[:])
            r1b2_sb = st.tile([128, 1], f32, tag="r1b2")
            nc.sync.dma_start(r1b2_sb[:], d_r1b2[:])
            r1b3_sb = st.tile([TARGET, 1], f32, tag="r1b3")
            nc.sync.dma_start(r1b3_sb[:], d_r1b3[:])
            r2b0_sb = st.tile([128, 1], f32, tag="r2b0")
            nc.sync.dma_start(r2b0_sb[:], d_r2b0[:])
            r2b1_sb = st.tile([128, 2], f32, tag="r2b1")
            nc.sync.dma_start(r2b1_sb[:], d_r2b1[:])
            r2b2_sb = st.tile([128, 1], f32, tag="r2b2")
            nc.sync.dma_start(r2b2_sb[:], d_r2b2[:])
            r2b3_sb = st.tile([TARGET, 1], f32, tag="r2b3")
            nc.sync.dma_start(r2b3_sb[:], d_r2b3[:])

            # ---- state ----
            h_all = stt.tile([N, G, H], f32, tag="h_all")
            nc.sync.dma_start(h_all[:], d_h0[:])
            hT_all = stt.tile([128, G, H], f32r, tag="hT_all")

            # ---- read mask (free-dim row over all graphs) ----
            colsum = ps.tile([1, G * N], f32, tag="pG2", padded_shape=None)
            for fb in range(FB):
                nc.tensor.matmul(
                    colsum[:, fb * GPB * N:(fb + 1) * GPB * N],
                    onescol_sb[:],
                    hT0_sb[:, fb * GPB:(fb + 1) * GPB, :],
                    start=True, stop=True)
            mask_row = st.tile([1, G * N], f32r, tag="mask_row")
            nc.vector.tensor_scalar(mask_row[:], colsum[:], 0.0, None,
                                    op0=ALU.not_equal)

            # ---- per-graph node masks [128,1] ----
            nmask = []
            for g in range(G):
                nmr = st.tile([N, 1], f32, tag=f"nmr{g}", name=f"nmr{g}")
                nc.vector.tensor_reduce(nmr[:], h_all[:, g, 0:F_IN], axis=AX.X,
                                        op=ALU.max, apply_absolute_value=True)
                nm = st.tile([N, 1], f32, tag=f"nm{g}", name=f"nm{g}")
                nc.vector.tensor_scalar(nm[:], nmr[:], 0.0, None,
                                        op0=ALU.not_equal)
                nmask.append(nm)

            # ---- message passing loop ----
            def hT_chunk(s, g, hc):
                if s == 0:
                    assert hc == 0
                    return hT0_sb[:, g, :]
                return hT_all[:, g, hc * 128:(hc + 1) * 128]

            for s in range(NSTEP):
                hcs = [0] if s == 0 else list(range(HC))
                for g in range(G):
                    # 1) per-label projections P_l = h @ A_l   [node, msg]
                    P_sb = []
                    for l in range(L):
                        pp = ps.tile([128, MSG], f32, tag="pP", bufs=2,
                                     name=f"pp_{s}_{g}_{l}")
                        for i, hc in enumerate(hcs):
                            nc.tensor.matmul(pp[:], hT_chunk(s, g, hc),
                                             A_sb[:, l, hc, :],
                                             start=(i == 0),
                                             stop=(i == len(hcs) - 1))
                        psb = wk.tile([128, MSG], f32r, tag="P", bufs=5,
                                      name=f"psb_{s}_{g}_{l}")
                        nc.vector.tensor_copy(psb[:], pp[:])
                        P_sb.append(psb)
                    # 2) aggregation m[v,:] = sum_l Ml^T.T @ P_l
                    mp = ps.tile([128, MSG], f32, tag="pM", bufs=1,
                                 name=f"mp_{s}_{g}")
                    for l in range(L):
                        nc.tensor.matmul(mp[:], mask_sb[:, g, l, :], P_sb[l][:],
                                         start=(l == 0), stop=(l == L - 1))
                    m_sb = wk.tile([128, MSG], f32, tag="m", bufs=2,
                                   name=f"m_{s}_{g}")
                    nc.vector.tensor_copy(m_sb[:], mp[:])
                    # 3) m^T via PE transposes
                    tp = ps.tile([128, MSG], f32, tag="pT", bufs=1,
                                 name=f"tp_{s}_{g}")
                    for c in range(MC):
                        nc.tensor.transpose(tp[:, c * 128:(c + 1) * 128],
                                            m_sb[:, c * 128:(c + 1) * 128],
                                            ident_sb[:])
                    mT_sb = wk.tile([128, MSG], f32r, tag="mT", bufs=2,
                                    name=f"mT_{s}_{g}")
                    nc.scalar.copy(mT_sb[:], tp[:])
                    # 4) GRU gate GEMMs (biases folded in via K=1 ones row)
                    grz = ps.tile([128, 2 * H], f32, tag="pG2", bufs=1,
                                  name=f"grz_{s}_{g}")
                    gin = ps.tile([128, H], f32, tag="pGi", bufs=1,
                                  name=f"gin_{s}_{g}")
                    ghn = ps.tile([128, H], f32, tag="pGh", bufs=1,
                                  name=f"ghn_{s}_{g}")
                    for half in range(2):
                        o = grz[:, half * H:(half + 1) * H]
                        mms = [(mT_sb[:, c * 128:(c + 1) * 128],
                                wih_sb[:, c, half * H:(half + 1) * H])
                               for c in range(MC)]
                        mms += [(hT_chunk(s, g, hc),
                                 whh_sb[:, hc, half * H:(half + 1) * H])
                                for hc in hcs]
                        mms.append((ones_sb[:],
                                    brz_sb[:, half * H:(half + 1) * H]))
                        for i, (lh, rh) in enumerate(mms):
                            nc.tensor.matmul(o, lh, rh, start=(i == 0),
                                             stop=(i == len(mms) - 1))
                    mms = [(mT_sb[:, c * 128:(c + 1) * 128],
                            wih_sb[:, c, 2 * H:3 * H]) for c in range(MC)]
                    mms.append((ones_sb[:], bin_sb[:]))
                    for i, (lh, rh) in enumerate(mms):
                        nc.tensor.matmul(gin[:], lh, rh, start=(i == 0),
                                         stop=(i == len(mms) - 1))
                    mms = [(hT_chunk(s, g, hc), whh_sb[:, hc, 2 * H:3 * H])
                           for hc in hcs]
                    mms.append((ones_sb[:], bhn_sb[:]))
                    for i, (lh, rh) in enumerate(mms):
                        nc.tensor.matmul(ghn[:], lh, rh, start=(i == 0),
                                         stop=(i == len(mms) - 1))
                    # 5) gate nonlinearities + state update
                    r_sb = wk.tile([128, H], f32, tag="r", bufs=2,
                                   name=f"r_{s}_{g}")
                    nc.scalar.activation(r_sb[:], grz[:, 0:H], AF.Sigmoid)
                    z_sb = wk.tile([128, H], f32, tag="z", bufs=2,
                                   name=f"z_{s}_{g}")
                    nc.scalar.activation(z_sb[:], grz[:, H:2 * H], AF.Sigmoid)
                    rhn = wk.tile([128, H], f32, tag="t1", bufs=2,
                                  name=f"rhn_{s}_{g}")
                    nc.vector.tensor_mul(rhn[:], r_sb[:], ghn[:])
                    npre = wk.tile([128, H], f32, tag="t2", bufs=2,
                                   name=f"npre_{s}_{g}")
                    nc.vector.tensor_add(npre[:], rhn[:], gin[:])
                    n_sb = wk.tile([128, H], f32, tag="n", bufs=2,
                                   name=f"n_{s}_{g}")
                    nc.scalar.activation(n_sb[:], npre[:], AF.Tanh)
                    d_t = wk.tile([128, H], f32, tag="t1", bufs=2,
                                  name=f"d_{s}_{g}")
                    nc.vector.tensor_sub(d_t[:], h_all[:, g, :], n_sb[:])
                    zd = wk.tile([128, H], f32, tag="t2", bufs=2,
                                 name=f"zd_{s}_{g}")
                    nc.vector.tensor_mul(zd[:], z_sb[:], d_t[:])
                    hnew = wk.tile([128, H], f32, tag="hnew", bufs=2,
                                   name=f"hnew_{s}_{g}")
                    nc.vector.tensor_add(hnew[:], n_sb[:], zd[:])
                    nc.vector.tensor_scalar_mul(h_all[:, g, :], hnew[:],
                                                nmask[g][:])
                    # 6) h^T for next step / readout
                    tp2 = ps.tile([128, H], f32, tag="pT", bufs=1,
                                  name=f"tp2_{s}_{g}")
                    for c in range(HC):
                        nc.tensor.transpose(tp2[:, c * 128:(c + 1) * 128],
                                            h_all[:, g, c * 128:(c + 1) * 128],
                                            ident_sb[:])
                    nc.scalar.copy(hT_all[:, g, :], tp2[:])

            # ---- readout ----
            out_sb = st.tile([TARGET, G], f32, tag="out_sb")
            for fb in range(FB):
                gsl = slice(fb * GPB, (fb + 1) * GPB)
                nfree = GPB * N  # 512

                def mlp_T(ws, bs, in_chunks, nlayers=4, pfx=""):
                    """Run an MLP in transposed layout, returning the last
                    pre-activation psum tile [out_dim, nfree].
                    ws/bs: weight/bias sb tiles per layer; in_chunks: list of
                    rhs APs for layer-0 contraction chunks."""
                    acts = in_chunks
                    # L0 -> 128
                    p = ps.tile([128, nfree], f32, tag="pP", bufs=2,
                                name=f"ro{pfx}p0_{fb}")
                    for i, (wap, rhs) in enumerate(zip(ws[0], acts)):
                        nc.tensor.matmul(p[:], wap, rhs, start=(i == 0),
                                         stop=(i == len(acts) - 1))
                    a1 = wk.tile([128, nfree], f32r, tag="P", bufs=5,
                                 name=f"ro{pfx}a1_{fb}")
                    nc.scalar.activation(a1[:], p[:], AF.Relu, bias=bs[0])
                    # L1 -> 256 (two 128-chunks)
                    a2 = []
                    for oc in range(2):
                        p2 = ps.tile([128, nfree], f32, tag="pP", bufs=2,
                                     name=f"ro{pfx}p1_{fb}_{oc}")
                        nc.tensor.matmul(p2[:], ws[1][oc], a1[:],
                                         start=True, stop=True)
                        t = wk.tile([128, nfree], f32r, tag="P", bufs=5,
                                    name=f"ro{pfx}a2_{fb}_{oc}")
                        nc.scalar.activation(t[:], p2[:], AF.Relu,
                                             bias=bs[1][oc])
                        a2.append(t)
                    # L2 -> 128
                    p3 = ps.tile([128, nfree], f32, tag="pP", bufs=2,
                                 name=f"ro{pfx}p2_{fb}")
                    for kc in range(2):
                        nc.tensor.matmul(p3[:], ws[2][kc], a2[kc][:],
                                         start=(kc == 0), stop=(kc == 1))
                    a3 = wk.tile([128, nfree], f32r, tag="P", bufs=5,
                                 name=f"ro{pfx}a3_{fb}")
                    nc.scalar.activation(a3[:], p3[:], AF.Relu, bias=bs[2])
                    # L3 -> TARGET
                    p4 = ps.tile([TARGET, nfree], f32, tag="pM", bufs=1,
                                 name=f"ro{pfx}p3_{fb}")
                    nc.tensor.matmul(p4[:], ws[3], a3[:], start=True, stop=True)
                    return p4

                r1_in = [hT_all[:, gsl, kc * 128:(kc + 1) * 128]
                         for kc in range(HC)] + [hT0_sb[:, gsl, :]]
                r1_ws = [[r1w0_sb[:, kc, :] for kc in range(5)],
                         [r1w1_sb[:, oc, :] for oc in range(2)],
                         [r1w2_sb[:, kc, :] for kc in range(2)],
                         r1w3_sb[:]]
                r1_bs = [r1b0_sb[:],
                         [r1b1_sb[:, oc:oc + 1] for oc in range(2)],
                         r1b2_sb[:]]
                p_gate = mlp_T(r1_ws, r1_bs, r1_in, pfx="g")
                gate_s = wk.tile([TARGET, nfree], f32, tag="r", bufs=2,
                                 name=f"gate_{fb}")
                nc.scalar.activation(gate_s[:], p_gate[:], AF.Sigmoid,
                                     bias=r1b3_sb[:])

                r2_in = [hT_all[:, gsl, kc * 128:(kc + 1) * 128]
                         for kc in range(HC)]
                r2_ws = [[r2w0_sb[:, kc, :] for kc in range(4)],
                         [r2w1_sb[:, oc, :] for oc in range(2)],
                         [r2w2_sb[:, kc, :] for kc in range(2)],
                         r2w3_sb[:]]
                r2_bs = [r2b0_sb[:],
                         [r2b1_sb[:, oc:oc + 1] for oc in range(2)],
                         r2b2_sb[:]]
                p_val = mlp_T(r2_ws, r2_bs, r2_in, pfx="v")
                val_s = wk.tile([TARGET, nfree], f32, tag="z", bufs=2,
                                name=f"val_{fb}")
                nc.scalar.activation(val_s[:], p_val[:], AF.Identity,
                                     bias=r2b3_sb[:])

                # mask broadcast [TARGET, nfree] via outer product
                mb = ps.tile([TARGET, nfree], f32, tag="pT", bufs=1,
                             name=f"mb_{fb}")
                nc.tensor.matmul(mb[:], ones12_sb[:],
                                 mask_row[:, fb * nfree:(fb + 1) * nfree],
                                 start=True, stop=True)
                pr = wk.tile([TARGET, nfree], f32, tag="t1", bufs=2,
                             name=f"pr_{fb}")
                nc.vector.tensor_mul(pr[:], gate_s[:], val_s[:])
                pr2 = wk.tile([TARGET, nfree], f32, tag="t2", bufs=2,
                              name=f"pr2_{fb}")
                nc.vector.tensor_mul(pr2[:], pr[:], mb[:])
                for gg in range(GPB):
                    ga = fb * GPB + gg
                    nc.vector.reduce_sum(out_sb[:, ga:ga + 1],
                                         pr2[:, gg * N:(gg + 1) * N],
                                         axis=AX.X)
            nc.sync.dma_start(d_out[:], out_sb[:])

    nc.compile()
    return nc


def _prep_core_inputs(core, g_, h_in, e, A, gru_Wih, gru_Whh, gru_bih,
                      gru_bhh, r1_Ws, r1_bs, r2_Ws, r2_bs):
    cs = slice(core * G, (core + 1) * G)
    f = np.float32
    h0 = np.zeros((G, N, H), f)
    h0[:, :, :F_IN] = h_in[cs]
    hT0 = np.ascontiguousarray(h_in[cs].transpose(2, 0, 1))  # [F, G, N]
    labels = np.arange(1, L + 1, dtype=f)
    # mask[w, g, l, v] = (e[g, v, w] == l+1)
    e_c = e[cs]  # [G, V, W]
    oh = (e_c[:, None, :, :] == labels[None, :, None, None]).astype(f)  # [G,L,V,W]
    mask = np.ascontiguousarray(oh.transpose(3, 0, 1, 2))  # [W, G, L, V]
    return {
        "h0": np.ascontiguousarray(h0.transpose(1, 0, 2)),  # [N, G, H]
        "hT0": hT0,
        "mask": mask,
    }


def _prep_shared_inputs(A, gru_Wih, gru_Whh, gru_bih, gru_bhh,
                        r1_Ws, r1_bs, r2_Ws, r2_bs):
    f = np.float32

    def chunk_rows(M, nch):  # [K, C] -> [128, nch, C] with K = nch*128
        K, C = M.shape
        assert K == nch * 128
        return np.ascontiguousarray(M.reshape(nch, 128, C).transpose(1, 0, 2))

    A_t = np.ascontiguousarray(
        A.reshape(L, HC, 128, MSG).transpose(2, 0, 1, 3))  # [128, L, HC, MSG]
    wih = chunk_rows(np.ascontiguousarray(gru_Wih.T), MC)   # [128, MC, 3H]
    whh = chunk_rows(np.ascontiguousarray(gru_Whh.T), HC)
    brz = (gru_bih + gru_bhh)[:2 * H].reshape(1, -1).astype(f)
    bin_ = gru_bih[2 * H:].reshape(1, -1).astype(f)
    bhn = gru_bhh[2 * H:].reshape(1, -1).astype(f)

    # readout weights, transposed layout
    r1w0t = np.ascontiguousarray(r1_Ws[0].T)  # [2H, 128]
    r1w0 = np.zeros((128, 5, 128), f)
    for kc in range(4):
        r1w0[:, kc, :] = r1w0t[kc * 128:(kc + 1) * 128]
    r1w0[:, 4, :] = r1w0t[H:H + F_IN]  # h0 chunk (features 0:128 of h0 half)
    r1w1 = np.ascontiguousarray(r1_Ws[1].T.reshape(128, 2, 128))
    r1w2 = chunk_rows(np.ascontiguousarray(r1_Ws[2].T), 2)
    r1w3 = np.ascontiguousarray(r1_Ws[3].T)  # [128, 12]
    r2w0 = chunk_rows(np.ascontiguousarray(r2_Ws[0].T), 4)
    r2w1 = np.ascontiguousarray(r2_Ws[1].T.reshape(128, 2, 128))
    r2w2 = chunk_rows(np.ascontiguousarray(r2_Ws[2].T), 2)
    r2w3 = np.ascontiguousarray(r2_Ws[3].T)

    return {
        "A": A_t, "wih": wih, "whh": whh,
        "brz": brz, "bin": bin_, "bhn": bhn,
        "ones": np.ones((1, N), f), "ones12": np.ones((1, TARGET), f),
        "onescol": np.ones((128, 1), f), "ident": np.eye(128, dtype=f),
        "r1w0": r1w0, "r1w1": r1w1, "r1w2": r1w2, "r1w3": r1w3,
        "r2w0": r2w0, "r2w1": r2w1, "r2w2": r2w2, "r2w3": r2w3,
        "r1b0": r1_bs[0].reshape(-1, 1).astype(f),
        "r1b1": np.ascontiguousarray(r1_bs[1].reshape(2, 128).T),
        "r1b2": r1_bs[2].reshape(-1, 1).astype(f),
        "r1b3": r1_bs[3].reshape(-1, 1).astype(f),
        "r2b0": r2_bs[0].reshape(-1, 1).astype(f),
        "r2b1": np.ascontiguousarray(r2_bs[1].reshape(2, 128).T),
        "r2b2": r2_bs[2].reshape(-1, 1).astype(f),
        "r2b3": r2_bs[3].reshape(-1, 1).astype(f),
    }


def _get_nc():
    if "nc" not in _CACHE:
        _CACHE["nc"] = _build()
    return _CACHE["nc"]


def _run(in_maps, **kwargs):
    nc = _get_nc()
    return run_bass_kernel_spmd(nc, in_maps, core_ids=list(range(NCORES)),
                                **kwargs)


def make_in_maps(g, h_in, e, A, gru_Wih, gru_Whh, gru_bih, gru_bhh,
                 r1_W0, r1_b0, r1_W1, r1_b1, r1_W2, r1_b2, r1_W3, r1_b3,
                 r2_W0, r2_b0, r2_W1, r2_b1, r2_W2, r2_b2, r2_W3, r2_b3):
    r1_Ws, r1_bs = [r1_W0, r1_W1, r1_W2, r1_W3], [r1_b0, r1_b1, r1_b2, r1_b3]
    r2_Ws, r2_bs = [r2_W0, r2_W1, r2_W2, r2_W3], [r2_b0, r2_b1, r2_b2, r2_b3]
    arrs = {k: np.asarray(v, np.float32) for k, v in dict(
        g=g, h_in=h_in, e=e, A=A, gru_Wih=gru_Wih, gru_Whh=gru_Whh,
        gru_bih=gru_bih, gru_bhh=gru_bhh).items()}
    r1_Ws = [np.asarray(w, np.float32) for w in r1_Ws]
    r1_bs = [np.asarray(b, np.float32) for b in r1_bs]
    r2_Ws = [np.asarray(w, np.float32) for w in r2_Ws]
    r2_bs = [np.asarray(b, np.float32) for b in r2_bs]
    shared = _prep_shared_inputs(arrs["A"], arrs["gru_Wih"], arrs["gru_Whh"],
                                 arrs["gru_bih"], arrs["gru_bhh"],
                                 r1_Ws, r1_bs, r2_Ws, r2_bs)
    in_maps = []
    for core in range(NCORES):
        m = dict(shared)
        m.update(_prep_core_inputs(core, arrs["g"], arrs["h_in"], arrs["e"],
                                   arrs["A"], arrs["gru_Wih"], arrs["gru_Whh"],
                                   arrs["gru_bih"], arrs["gru_bhh"],
                                   r1_Ws, r1_bs, r2_Ws, r2_bs))
        in_maps.append(m)
    return in_maps


def kernel(**inputs):
    in_maps = make_in_maps(**inputs)
    res = _run(in_maps)
    out = np.zeros((B, TARGET), np.float32)
    for core in range(NCORES):
        out[core * G:(core + 1) * G] = res.results[core]["out"].T
    return out


if __name__ == "__main__":
    import reference
    inputs = {k: np.asarray(v) for k, v in reference.setup_inputs().items()}
    expected = np.asarray(reference.reference(**inputs))
    actual = kernel(**inputs)
    scale = np.abs(expected).max()
    err = np.abs(actual - expected).max() / scale
    print("Relative error:", err)
